# revision 23
# baseline (speedup 1.0000x reference)
"""Trainium2 Bass kernel v3 for the 3-layer PointGNN-style edge-scored GNN.

Design vs v2:
  - node phases for layers 2/3 are emitted bin-by-bin INSIDE the previous
    layer's edge loop (post_emit), hiding their matmuls/stores under the
    DVE-bound edge pipeline instead of running as serial blocks.
  - L1/L2 edge scoring batched per bin: one TT add (bf16, 2x DVE mode), one
    scalar_tensor_tensor relu*w2 over all nt_e tiles, one tensor_reduce
    (replaces 17 per-tile stt ops). Message scaling stays on the ACT engine
    (idle during edges; a DVE broadcast-mult would run at 1x rate on the
    bottleneck engine).
  - L1/L2 and L3 gather buffers share SBUF slots (one flat [P, CB12*nt_e*P]
    pair viewed per-layer) instead of co-allocating, freeing 34KB/partition.
  - AG_MODE="chunked" (optional): chunk-major table rows so each per-layer
    AllGather splits into NCH contiguous-slice collectives overlapped with
    the edge pipeline. Measured neutral vs one AG per layer on HW (cost is
    within session noise), so "real" stays the default.
  - v2 retained: sharded node phase + per-layer bf16 table AllGather;
    dst-A from a local a_tab gather; L3 transposed gathers with PE-transpose
    scatter accumulating G^T = sum_e s_e*[h2_e|1] then one G^T.T @ W3x.
"""

import sys

if "/opt/trn_rl_repo" not in sys.path:
    sys.path.insert(0, "/opt/trn_rl_repo")

import numpy as np
import ml_dtypes

import concourse.bacc as bacc
import concourse.bass as bass  # noqa: F401
import concourse.mybir as mybir
import concourse.tile as tile
from concourse.bass_utils import run_bass_kernel_spmd

F32 = mybir.dt.float32
BF16 = mybir.dt.bfloat16
I16 = mybir.dt.int16
AF = mybir.ActivationFunctionType
ALU = mybir.AluOpType
BF = ml_dtypes.bfloat16

P = 128
NCORES = 8
SIM_MODE = False
# "chunked": table rows are chunk-major; each layer's AllGather is split into
#   NCH collectives emitted as soon as their third of the node phase is done,
#   overlapping the collective with the previous layer's edge pipeline.
# "real": single AllGather per layer (core-major rows).
# "none": collectives disabled (timing experiments only — results invalid).
AG_MODE = "real"
CHB = 10          # bins per AG chunk
NCH = 3           # chunks (= nbc / CHB)
CB12 = 4  # bins per gather chunk, layers 1/2
CB3 = 2   # bins per gather chunk, layer 3


class Cfg:
    def __init__(self, n_real, nbc, dims):
        self.n_real = n_real
        self.nbc = nbc
        self.nb = nbc * NCORES
        self.ng = self.nb * P
        self.dims = dims


CFG = Cfg(30000, 30, [(256, 64), (64, 64), (64, 256)])


# ---------------------------------------------------------------- host prep

def _balance_bins(weight, nb):
    import heapq

    n = weight.shape[0]
    order = np.argsort(-weight, kind="stable")
    bin_of = np.empty(n, np.int32)
    slot_of = np.empty(n, np.int32)
    counts = np.zeros(nb, np.int32)
    heap = [(0, b) for b in range(nb)]
    heapq.heapify(heap)
    for i in order:
        spill = []
        while True:
            load, b = heapq.heappop(heap)
            if counts[b] < P:
                break
            spill.append((load, b))
        for s in spill:
            heapq.heappush(heap, s)
        bin_of[i] = b
        slot_of[i] = counts[b]
        counts[b] += 1
        heapq.heappush(heap, (load + int(weight[i]), b))
    return bin_of, slot_of


def _wrap16(flat_idx):
    n = flat_idx.shape[0]
    a = flat_idx.reshape(n // 16, 16).T.astype(np.int16)
    return np.tile(a, (8, 1))


def _host_prep(x, src, dst, cfg):
    n = cfg.n_real
    loops = np.arange(n, dtype=np.int64)
    src_all = np.concatenate([src, loops])
    dst_all = np.concatenate([dst, loops])

    indeg = np.bincount(dst_all, minlength=n).astype(np.int64)
    bin_of, slot_of = _balance_bins(indeg, cfg.nb)
    g_of = bin_of.astype(np.int64) * P + slot_of

    e_bin = bin_of[dst_all]
    order = np.argsort(e_bin, kind="stable")
    sb = e_bin[order]
    counts = np.bincount(e_bin, minlength=cfg.nb)
    g_pad = int(np.ceil(max(counts.max(), 1) / P) * P)
    starts = np.zeros(cfg.nb, np.int64)
    starts[1:] = np.cumsum(counts)[:-1]
    rank = np.arange(sb.shape[0]) - starts[sb]

    # Table-row id for AllGathered tables. "chunked": chunk-major layout so
    # each per-layer AG splits into NCH contiguous-slice collectives:
    #   row = chunk*(8*CHB*P) + core*(CHB*P) + (t%CHB)*P + slot
    # "real"/"none": core-major (bin*P + slot).
    if AG_MODE == "chunked":
        c_all = bin_of // cfg.nbc
        t_all = bin_of % cfg.nbc
        tblrow_of = (
            (t_all // CHB).astype(np.int64) * (NCORES * CHB * P)
            + c_all.astype(np.int64) * (CHB * P)
            + (t_all % CHB).astype(np.int64) * P
            + slot_of
        )
    else:
        tblrow_of = g_of

    src_g = np.zeros((cfg.nb, g_pad), np.int64)             # pad edges -> row 0
    dst_slot = np.full((cfg.nb, g_pad), 255, np.int64)      # pad -> no match
    src_g[sb, rank] = tblrow_of[src_all[order]]
    dst_slot[sb, rank] = slot_of[dst_all[order]]

    nt_e = g_pad // P
    per_core = []
    for c in range(NCORES):
        bins = slice(c * cfg.nbc, (c + 1) * cfg.nbc)
        sg = src_g[bins]
        ds = dst_slot[bins]
        srcw = np.concatenate([_wrap16(sg[t]) for t in range(cfg.nbc)], axis=1)
        # local A-table row index: t_local*128 + slot (pads -> 0)
        aidx = np.arange(cfg.nbc)[:, None] * P + np.where(ds == 255, 0, ds)
        dstw = np.concatenate([_wrap16(aidx[t]) for t in range(cfg.nbc)], axis=1)
        tloc = aidx // P
        slt = aidx % P
        if AG_MODE == "chunked":
            gidx = (
                (tloc // CHB) * (NCORES * CHB * P)
                + c * (CHB * P)
                + (tloc % CHB) * P
                + slt
            )
        else:
            gidx = aidx + c * cfg.nbc * P
        dstwg = np.concatenate([_wrap16(gidx[t]) for t in range(cfg.nbc)], axis=1)
        dstc = np.concatenate(
            [ds[t].reshape(nt_e, P).T for t in range(cfg.nbc)], axis=1
        ).astype(BF)
        per_core.append((srcw, dstw, dstwg, dstc))

    # own-node features, feature-major, per core: [256, n_loc] bf16
    c_in = cfg.dims[0][0]
    x1t = np.zeros((c_in, cfg.ng), BF)
    x1t[:, g_of] = x.T.astype(BF)
    return g_of, g_pad, per_core, x1t


def _fuse_weights(ws, cfg):
    """Per layer: wmat [ci, 3co] cols = [x'|B|A], brep biases [3co];
    L3 separate: w3B/w3A [64,256]+biases, w3x_aug [65,256] (row 64 = b_lin3)."""
    out = []
    for li, (ci, co) in enumerate(cfg.dims, start=1):
        wl = ws[f"w_lin{li}"].astype(np.float64)
        bl = ws[f"b_lin{li}"].astype(np.float64)
        ws1 = ws[f"w_s1_{li}"].astype(np.float64)
        bs1 = ws[f"b_s1_{li}"].astype(np.float64)
        ws2 = ws[f"w_s2_{li}"].astype(np.float64)
        bs2 = ws[f"b_s2_{li}"].astype(np.float64)
        wi, wj = ws1[:co], ws1[co:]
        wmat = np.zeros((ci, 3 * co), np.float64)
        bias = np.zeros((3 * co,), np.float64)
        wmat[:, :co] = wl
        bias[:co] = bl
        wmat[:, co : 2 * co] = wl @ wj
        bias[co : 2 * co] = bl @ wj
        wmat[:, 2 * co :] = wl @ wi
        bias[2 * co :] = bl @ wi + bs1
        out.append(
            dict(
                wmat=wmat.astype(BF),
                bias=bias.astype(np.float32),
                w2=ws2[:, 0].astype(np.float32),
                b2=np.float32(bs2[0]),
                wlin=wl,
                blin=bl,
            )
        )
    return out


# ---------------------------------------------------------------- program

def _build_program(cfg, g_pad):
    nbc, ng = cfg.nbc, cfg.ng
    nt_e = g_pad // P
    dims = cfg.dims
    n_loc = nbc * P
    co3 = dims[2][1]  # 256

    # bf16 const blob columns
    c_iota, c_ident = 0, 128
    c_w2 = [256, 256 + 64, 256 + 128]  # w2 reps: 64,64,256
    c_brep1 = 256 + 128 + 256          # 192
    c_brep2 = c_brep1 + 192            # 192
    c_brep3b = c_brep2 + 192           # 256 (B3 bias rep)
    c_brep3a = c_brep3b + 256          # 256 (A3 bias rep)
    c_iotarep = c_brep3a + 256         # 128*nt_e: [p, s*nt_e+j] = s
    cb_cols = c_iotarep + P * nt_e

    nc = bacc.Bacc("TRN2", target_bir_lowering=False, debug=False, num_devices=NCORES,
                   dynamic_dma_scratch_size=32768)

    # All bf16/f32 constants + streamed x features packed into ONE input
    # tensor, and the three wrapped index tables into a second one: per-call
    # PJRT buffer bindings through the axon tunnel cost ~15-20us each, so
    # 13 inputs -> 2 shaves ~0.2 ms/exec. Offsets mirrored in _make_in_maps.
    o_dstc = cb_cols
    o_w1 = o_dstc + nbc * nt_e
    o_w2m = o_w1 + 2 * 192
    o_w3b = o_w2m + 192
    o_w3a = o_w3b + co3
    o_w3x = o_w3a + co3
    o_cstf = o_w3x + co3
    o_xa = o_cstf + 8          # 4 f32 bias cols bitcast as 8 bf16 cols
    nb_cols = o_xa + 2 * n_loc
    iw = nbc * g_pad // 16     # cols per wrapped index table

    blob_d = nc.dram_tensor("blob", [P, nb_cols], BF16, kind="ExternalInput")
    idx_d = nc.dram_tensor("idx", [P, 3 * iw], I16, kind="ExternalInput")
    out_d = nc.dram_tensor("out", [n_loc, co3], F32, kind="ExternalOutput")

    with tile.TileContext(nc) as tc:
        with (
            tc.tile_pool(name="cst", bufs=1) as cpool,
            tc.tile_pool(name="persist", bufs=1) as ppool,
            tc.tile_pool(name="xa", bufs=2) as xapool,
            tc.tile_pool(name="wrt", bufs=2) as wpool,
            tc.tile_pool(name="stg", bufs=3) as spool,
            tc.tile_pool(name="gath", bufs=2) as gpool,
            tc.tile_pool(name="oh", bufs=3) as opool,
            tc.tile_pool(name="sc", bufs=2) as scpool,
            tc.tile_pool(name="psA", bufs=2, space="PSUM") as pspA,
            tc.tile_pool(name="psB", bufs=2, space="PSUM") as pspB,
            tc.tile_pool(name="dram", bufs=1, space="DRAM") as dpool,
        ):
            # ---------------- constants / persistent inputs
            # one SBUF blob holding every persistent constant; named slices
            # below are views into it (xa region streams separately)
            cst = cpool.tile([P, o_xa], BF16)
            nc.sync.dma_start(cst[:], blob_d[:, 0:o_xa])
            idxt = cpool.tile([P, 3 * iw], I16)
            nc.sync.dma_start(idxt[:], idx_d[:])

            cstb = cst[:, 0:cb_cols]
            dstc = cst[:, o_dstc : o_dstc + nbc * nt_e]
            w1 = cst[:, o_w1 : o_w1 + 2 * 192].rearrange("p (c n) -> p c n", c=2)
            w2m = cst[0:64, o_w2m : o_w2m + 192]
            w3b = cst[:, o_w3b : o_w3b + co3]
            w3a = cst[:, o_w3a : o_w3a + co3]
            w3x = cst[0:65, o_w3x : o_w3x + co3]
            cstf = cst[:, o_cstf : o_cstf + 8].bitcast(F32)
            srcw = idxt[:, 0:iw]
            dstw = idxt[:, iw : 2 * iw]
            dstwg = idxt[:, 2 * iw : 3 * iw]

            iota_row = cstb[:, c_iota : c_iota + P]      # [P,128], row-iota
            identb = cstb[:, c_ident : c_ident + P]      # [P,128] identity bf16

            # xloc: per-layer own output, feature-major, row 64 = ones
            xloc = [ppool.tile([65, n_loc], BF16, tag=f"xloc{l}", name=f"xloc{l}") for l in range(2)]
            nc.vector.memset(xloc[1][64:65, :], 1.0)

            # ---------------- DRAM internals
            tbl = [
                dpool.tile([ng, P], BF16, tag=f"tbl{l}", name=f"tbl{l}",
                           addr_space="Local" if (SIM_MODE or AG_MODE == "chunked") else "Shared")
                for l in range(2)
            ]
            tbl_loc = [dpool.tile([n_loc, P], BF16, tag=f"tbll{l}", name=f"tbll{l}") for l in range(2)]
            hb3 = dpool.tile([ng, P], BF16, tag="hb3", name="hb3",
                             addr_space="Local" if (SIM_MODE or AG_MODE == "chunked") else "Shared")
            hb3_loc = dpool.tile([n_loc, P], BF16, tag="hb3l", name="hb3l")
            atab = [dpool.tile([n_loc, P], BF16, tag=f"atab{l}", name=f"atab{l}") for l in range(2)]

            def allgather(loc, full):
                if AG_MODE == "none":
                    return
                if SIM_MODE:
                    rows = loc.shape[0]
                    for r in range(NCORES):
                        nc.sync.dma_start(full[:][r * rows : (r + 1) * rows, :], loc[:])
                else:
                    nc.gpsimd.collective_compute(
                        "AllGather",
                        ALU.bypass,
                        replica_groups=[list(range(NCORES))],
                        ins=[loc.opt()],
                        outs=[full.opt()],
                    )

            def allgather_chunk(loc, full, k):
                """AG of node-bin chunk k: contiguous row slices on both sides."""
                if AG_MODE == "none":
                    return
                rows = CHB * P
                in_ap = loc[:][k * rows : (k + 1) * rows, :]
                out_ap = full[:][k * NCORES * rows : (k + 1) * NCORES * rows, :]
                if SIM_MODE:
                    for r in range(NCORES):
                        nc.sync.dma_start(
                            full[:][k * NCORES * rows + r * rows :][0:rows, :], in_ap
                        )
                else:
                    nc.gpsimd.collective_compute(
                        "AllGather",
                        ALU.bypass,
                        replica_groups=[list(range(NCORES))],
                        ins=[in_ap],
                        outs=[out_ap],
                    )

            # ================ node phase layer l (own bins): tables + a_tab
            # make_node_emitter(l) returns emit(t): emits bin t's node work.
            # Called standalone (node_phase) or interleaved into the previous
            # layer's edge loop so the node matmuls/stores hide under it.
            def make_node_emitter(l):
                co = dims[l][1]
                state = {}

                def emit(t):
                    cols = slice(t * P, (t + 1) * P)
                    if l == 2:
                        ps_h = pspB.tile([P, P], F32, space="PSUM", tag="g")
                        nc.tensor.matmul(out=ps_h[:], lhsT=xloc[1][:, cols], rhs=identb[0:65, :], start=True, stop=True)
                        hbs = spool.tile([P, P], BF16, tag="hbs")
                        nc.vector.tensor_copy(out=hbs[:], in_=ps_h[:])
                        nc.sync.dma_start(hb3_loc[:][t * P : (t + 1) * P, :], hbs[:])
                        return
                    ps = pspA.tile([P, 192], F32, space="PSUM", tag="big")
                    if l == 0:
                        XB = 10
                        if t % XB == 0:
                            state["xa"] = xapool.tile([P, 2, XB * P], BF16, tag="xa", name="xa")
                            gcols = slice(t * P, min((t + XB) * P, n_loc))
                            nc.sync.dma_start(
                                state["xa"][:, :, 0 : gcols.stop - gcols.start],
                                blob_d[:, o_xa : o_xa + 2 * n_loc]
                                .rearrange("p (c n) -> p c n", c=2)[:, :, gcols],
                            )
                        lc = slice((t % XB) * P, (t % XB + 1) * P)
                        for k in range(2):
                            nc.tensor.matmul(
                                out=ps[:], lhsT=state["xa"][:, k, lc], rhs=w1[:, k, :],
                                start=(k == 0), stop=(k == 1),
                            )
                    else:
                        nc.tensor.matmul(out=ps[:], lhsT=xloc[0][0:64, cols], rhs=w2m[:], start=True, stop=True)
                    brep = cstb[:, (c_brep1 if l == 0 else c_brep2) :][:, 0:192]
                    WB = 5
                    g = t % WB
                    if g == 0:
                        state["tbs"] = wpool.tile([P, WB, P], BF16, tag="tbs", name="tbs")
                        state["ats"] = wpool.tile([P, WB, P], BF16, tag="ats", name="ats")
                    tbs, ats = state["tbs"], state["ats"]
                    nc.vector.tensor_tensor(out=tbs[:, g, :], in0=ps[:, 0:P], in1=brep[:, 0:P], op=ALU.add)
                    nc.vector.tensor_tensor(
                        out=ats[:, g, 0:co], in0=ps[:, 2 * co : 3 * co],
                        in1=brep[:, 2 * co : 3 * co], op=ALU.add,
                    )
                    nc.vector.memset(ats[:, g, co:P], 0.0)
                    if g == WB - 1:
                        t0 = t - WB + 1
                        nc.sync.dma_start(
                            tbl_loc[l][:][t0 * P : (t + 1) * P, :]
                            .rearrange("(g p) n -> p g n", p=P),
                            tbs[:],
                        )
                        nc.sync.dma_start(
                            atab[l][:][t0 * P : (t + 1) * P, :]
                            .rearrange("(g p) n -> p g n", p=P),
                            ats[:],
                        )

                return emit

            def node_phase(l, ag=None):
                emit = make_node_emitter(l)
                for t in range(nbc):
                    emit(t)
                    if ag is not None and t % CHB == CHB - 1:
                        allgather_chunk(*ag, t // CHB)

            # ================ edge phase layer l (own bins)
            def edge_phase(l, post_emit=None, ag=None):
                co = dims[l][1]
                w2rep = cstb[:, c_w2[l] : c_w2[l] + co]
                b2col = cstf[:, l : l + 1]
                cb = CB3 if l == 2 else CB12
                nch = (nbc + cb - 1) // cb
                gs = ga = None
                pend = None

                def emit_b3(tb, gstb, tlb, ssigb, ohb):
                    gt_ps = pspB.tile([65, P], F32, space="PSUM", tag="g")

                    def tr_mm(j):
                        esl = slice(tlb * g_pad + j * P, tlb * g_pad + (j + 1) * P)
                        tr = pspB.tile([P, P], F32, space="PSUM", tag="scat")
                        nc.tensor.matmul(out=tr[:], lhsT=gstb[:, 0, esl], rhs=identb, start=True, stop=True)
                        return tr

                    trs = [tr_mm(0), tr_mm(1)]
                    for j in range(nt_e):
                        hs = opool.tile([P, P], BF16, tag="sm")
                        nc.scalar.activation(
                            out=hs[:], in_=trs[j][:], func=AF.Copy,
                            scale=ssigb[:, j : j + 1],
                        )
                        nc.tensor.matmul(
                            out=gt_ps[:], lhsT=hs[:, 0:65], rhs=ohb[:, :, j],
                            start=(j == 0), stop=(j == nt_e - 1),
                        )
                        if j + 2 < nt_e:
                            trs.append(tr_mm(j + 2))
                    gt_sb = spool.tile([65, P], BF16, tag="gt_sb")
                    nc.vector.tensor_copy(out=gt_sb[:], in_=gt_ps[:])
                    o_ps = pspA.tile([P, co3], F32, space="PSUM", tag="obig")
                    nc.tensor.matmul(out=o_ps[:], lhsT=gt_sb[:], rhs=w3x[:], start=True, stop=True)
                    ostg = spool.tile([P, co3], F32, tag="ostg")
                    nc.scalar.activation(out=ostg[:], in_=o_ps[:], func=AF.Copy)
                    nc.sync.dma_start(out_d[tb * P : (tb + 1) * P, :], ostg[:])

                for t in range(nbc):
                    if t % cb == 0:
                        hn = min(cb, nbc - t)
                        ni = hn * g_pad
                        isl = slice(t * g_pad // 16, (t + hn) * g_pad // 16)
                        # unified gather buffers: one flat [P, CB12*nt_e*P] slot
                        # per stream, viewed per-layer (L1/L2 row-major chunks,
                        # L3 transposed edge-major) so L3 doesn't co-allocate.
                        gbuf0 = gpool.tile([P, CB12 * nt_e * P], BF16, tag="g0", name="gbuf0")
                        gbuf1 = gpool.tile([P, CB12 * nt_e * P], BF16, tag="g1", name="gbuf1")
                        if l == 2:
                            gst = gbuf0[:, 0 : CB3 * g_pad].rearrange("p (o e) -> p o e", o=1)
                            gdt = gbuf1[:, 0 : CB3 * g_pad].rearrange("p (o e) -> p o e", o=1)
                            nc.gpsimd.dma_gather(
                                out_ap=gst[:, :, 0:ni], in_ap=hb3[:],
                                idxs_ap=srcw[:, isl], num_idxs=ni, num_idxs_reg=ni,
                                elem_size=P, transpose=True, single_packet=False,
                            )
                            nc.gpsimd.dma_gather(
                                out_ap=gdt[:, :, 0:ni], in_ap=hb3[:],
                                idxs_ap=dstwg[:, isl], num_idxs=ni, num_idxs_reg=ni,
                                elem_size=P, transpose=True, single_packet=False,
                            )
                        else:
                            gs = gbuf0[:].rearrange("p (j e) -> p j e", e=P)
                            ga = gbuf1[:].rearrange("p (j e) -> p j e", e=P)
                            nc.gpsimd.dma_gather(
                                out_ap=ga[:, 0 : hn * nt_e, :], in_ap=atab[l][:],
                                idxs_ap=dstw[:, isl], num_idxs=ni, num_idxs_reg=ni,
                                elem_size=P, single_packet=False,
                            )
                            nc.gpsimd.dma_gather(
                                out_ap=gs[:, 0 : hn * nt_e, :], in_ap=tbl[l][:],
                                idxs_ap=srcw[:, isl], num_idxs=ni, num_idxs_reg=ni,
                                elem_size=P, single_packet=False,
                            )
                    tl = t % cb
                    g3 = None if l == 2 else gs[:, tl * nt_e : (tl + 1) * nt_e, :]
                    spre = scpool.tile([P, nt_e], F32, tag="spre")
                    if l == 2:
                        for j in range(nt_e):
                            esl = slice(tl * g_pad + j * P, tl * g_pad + (j + 1) * P)
                            ab_ps = pspA.tile([P, co3], F32, space="PSUM", tag="big")
                            nc.tensor.matmul(out=ab_ps[:], lhsT=gst[:, 0, esl], rhs=w3b[:], start=True, stop=False)
                            nc.tensor.matmul(out=ab_ps[:], lhsT=gdt[:, 0, esl], rhs=w3a[:], start=False, stop=True)
                            scr = scpool.tile([P, co3], F32, tag="scr")
                            nc.vector.scalar_tensor_tensor(
                                out=scr[:], in0=ab_ps[:], scalar=0.0,
                                in1=w2rep, op0=ALU.max, op1=ALU.mult,
                                accum_out=spre[:, j : j + 1],
                            )
                    else:
                        a3 = ga[:, tl * nt_e : (tl + 1) * nt_e, :]
                        ssum = scpool.tile([P, nt_e, co], BF16, tag="ssum")
                        nc.vector.tensor_tensor(
                            out=ssum[:], in0=g3[:, :, 64:P], in1=a3[:, :, 0:co], op=ALU.add,
                        )
                        # batched relu*w2 over all nt_e tiles, then segmented sum
                        scr = scpool.tile([P, nt_e, co], F32, tag="scr")
                        nc.vector.scalar_tensor_tensor(
                            out=scr[:], in0=ssum[:], scalar=0.0,
                            in1=w2rep.rearrange("p (o c) -> p o c", o=1)
                            .to_broadcast([P, nt_e, co]),
                            op0=ALU.max, op1=ALU.mult,
                        )
                        nc.vector.tensor_reduce(
                            out=spre[:], in_=scr[:], axis=mybir.AxisListType.X, op=ALU.add,
                        )
                    ssig = scpool.tile([P, nt_e], F32, tag="ssig")
                    nc.scalar.activation(out=ssig[:], in_=spre[:], func=AF.Sigmoid, bias=b2col)

                    # one-hot [e, slot, j] (packed operands -> DVE 2x mode)
                    oh = opool.tile([P, P, nt_e], BF16, tag="oh")
                    nc.vector.tensor_tensor(
                        out=oh[:],
                        in0=dstc[:, t * nt_e : (t + 1) * nt_e]
                        .rearrange("p (o j) -> p o j", o=1).to_broadcast([P, P, nt_e]),
                        in1=cstb[:, c_iotarep : c_iotarep + P * nt_e].rearrange(
                            "p (s j) -> p s j", s=P),
                        op=ALU.is_equal,
                    )

                    if l == 2:
                        if pend is not None:
                            emit_b3(*pend)
                        pend = (t, gst, tl, ssig, oh)
                    else:
                        # per-j score scaling on ACT (idle engine; DVE is the
                        # edge-phase bottleneck and broadcast muls run at 1x)
                        s_ps = pspB.tile([64, P], F32, space="PSUM", tag="scat")
                        for j in range(nt_e):
                            msg = opool.tile([P, co], BF16, tag="sm")
                            nc.scalar.activation(
                                out=msg[:], in_=g3[:, j, 0:co], func=AF.Copy,
                                scale=ssig[:, j : j + 1],
                            )
                            nc.tensor.matmul(
                                out=s_ps[:], lhsT=msg[:], rhs=oh[:, :, j],
                                start=(j == 0), stop=(j == nt_e - 1),
                            )
                        nc.scalar.activation(
                            out=xloc[l][0:64, t * P : (t + 1) * P], in_=s_ps[:], func=AF.Relu
                        )
                    if post_emit is not None:
                        post_emit(t)
                        if ag is not None and t % CHB == CHB - 1:
                            allgather_chunk(*ag, t // CHB)
                if l == 2 and pend is not None:
                    emit_b3(*pend)

            # ================ schedule
            # node phases for layers 2/3 are interleaved bin-by-bin into the
            # previous layer's edge loop so they hide under the edge pipeline.
            if AG_MODE == "chunked":
                node_phase(0, ag=(tbl_loc[0], tbl[0]))
                edge_phase(0, post_emit=make_node_emitter(1), ag=(tbl_loc[1], tbl[1]))
                edge_phase(1, post_emit=make_node_emitter(2), ag=(hb3_loc, hb3))
                edge_phase(2)
            else:
                node_phase(0)
                allgather(tbl_loc[0], tbl[0])
                edge_phase(0, post_emit=make_node_emitter(1))
                allgather(tbl_loc[1], tbl[1])
                edge_phase(1, post_emit=make_node_emitter(2))
                allgather(hb3_loc, hb3)
                edge_phase(2)

    nc.compile()
    return nc


# ---------------------------------------------------------------- driver

_PROG_CACHE = {}


def _make_in_maps(inputs, cfg, g_pad, per_core, x1t, fw):
    nbc = cfg.nbc
    n_loc = nbc * P
    co3 = cfg.dims[2][1]

    nt_e = g_pad // P
    cb_cols = 256 + 128 + 256 + 192 + 192 + 256 + 256 + P * nt_e
    cstb = np.zeros((P, cb_cols), BF)
    cstb[:, 0:128] = np.arange(128, dtype=np.float32)[None, :].astype(BF)
    cstb[:, 128:256] = np.eye(128, dtype=np.float32).astype(BF)
    off = 256
    for l in range(3):
        co = cfg.dims[l][1]
        cstb[:, off + (0 if l == 0 else (64 if l == 1 else 128)) :][:, 0:co] = (
            fw[l]["w2"][None, :].astype(BF)
        )
    off = 256 + 128 + 256
    cstb[:, off : off + 192] = fw[0]["bias"][None, :].astype(BF)
    cstb[:, off + 192 : off + 384] = fw[1]["bias"][None, :].astype(BF)
    cstb[:, off + 384 : off + 384 + 256] = fw[2]["bias"][None, co3 : 2 * co3].astype(BF)
    cstb[:, off + 640 : off + 640 + 256] = fw[2]["bias"][None, 2 * co3 : 3 * co3].astype(BF)
    nt_e = g_pad // P
    c_iotarep = off + 640 + 256
    cstb[:, c_iotarep : c_iotarep + P * nt_e] = np.repeat(
        np.arange(P, dtype=np.float32), nt_e
    )[None, :].astype(BF)

    cstf = np.zeros((P, 4), np.float32)
    for l in range(3):
        cstf[:, l] = fw[l]["b2"]

    w1 = np.zeros((2, P, 192), BF)
    w1[0] = fw[0]["wmat"][0:128]
    w1[1] = fw[0]["wmat"][128:256]
    w2m = np.ascontiguousarray(fw[1]["wmat"])
    w3b = np.zeros((P, co3), BF)
    w3b[0:64] = fw[2]["wmat"][:, co3 : 2 * co3]
    w3b[64] = fw[2]["bias"][co3 : 2 * co3].astype(BF)
    w3a = np.zeros((P, co3), BF)
    w3a[0:64] = fw[2]["wmat"][:, 2 * co3 : 3 * co3]
    w3a[64] = fw[2]["bias"][2 * co3 : 3 * co3].astype(BF)
    w3x = np.zeros((65, co3), BF)
    w3x[0:64] = fw[2]["wlin"].astype(BF)
    w3x[64] = fw[2]["blin"].astype(BF)

    # pack into the two-blob layout (offset order mirrored in _build_program:
    # cstb | dstc | w1 | w2m | w3b | w3a | w3x | cstf(bitcast) | xa)
    w1p = np.concatenate([w1[0], w1[1]], axis=1)            # [P, 384]
    w2mp = np.zeros((P, 192), BF)
    w2mp[0:64] = w2m
    w3xp = np.zeros((P, co3), BF)
    w3xp[0:65] = w3x
    cstf_b = np.ascontiguousarray(cstf).view(BF)            # [P, 8]

    in_maps = []
    for c in range(NCORES):
        srcw, dstw, dstwg, dstc = per_core[c]
        sl = slice(c * n_loc, (c + 1) * n_loc)
        xa_p = np.concatenate([x1t[0:128, sl], x1t[128:256, sl]], axis=1)
        blob = np.ascontiguousarray(
            np.concatenate(
                [cstb, dstc, w1p, w2mp, w3b, w3a, w3xp, cstf_b, xa_p], axis=1
            )
        )
        idx = np.ascontiguousarray(np.concatenate([srcw, dstw, dstwg], axis=1))
        in_maps.append({"blob": blob, "idx": idx})
    return in_maps


def _run(inputs, cfg, trace=False):
    x = np.ascontiguousarray(np.asarray(inputs["x"], dtype=np.float32))
    ei = np.asarray(inputs["edge_index"]).astype(np.int64)
    src, dst = ei[0], ei[1]

    g_of, g_pad, per_core, x1t = _host_prep(x, src, dst, cfg)
    fw = _fuse_weights(inputs, cfg)

    key = (cfg.n_real, cfg.nbc, g_pad)
    if key not in _PROG_CACHE:
        _PROG_CACHE[key] = _build_program(cfg, g_pad)
    nc = _PROG_CACHE[key]

    in_maps = _make_in_maps(inputs, cfg, g_pad, per_core, x1t, fw)
    res = run_bass_kernel_spmd(nc, in_maps, core_ids=list(range(NCORES)), trace=trace)

    n_loc = cfg.nbc * P
    full = np.empty((cfg.ng, cfg.dims[-1][1]), np.float32)
    for c in range(NCORES):
        full[c * n_loc : (c + 1) * n_loc] = res.results[c]["out"]
    out = full[g_of]
    return out, res


def kernel(**inputs) -> np.ndarray:
    out, _ = _run(inputs, CFG, trace=False)
    return out



# revision 26
# speedup vs baseline: 1.1243x; 1.1243x over previous
"""Trainium2 Bass kernel v3 for the 3-layer PointGNN-style edge-scored GNN.

Design vs v2:
  - node phases for layers 2/3 are emitted bin-by-bin INSIDE the previous
    layer's edge loop (post_emit), hiding their matmuls/stores under the
    DVE-bound edge pipeline instead of running as serial blocks.
  - L1/L2 edge scoring batched per bin: one TT add (bf16, 2x DVE mode), one
    scalar_tensor_tensor relu*w2 over all nt_e tiles, one tensor_reduce
    (replaces 17 per-tile stt ops). Message scaling stays on the ACT engine
    (idle during edges; a DVE broadcast-mult would run at 1x rate on the
    bottleneck engine).
  - L1/L2 and L3 gather buffers share SBUF slots (one flat [P, CB12*nt_e*P]
    pair viewed per-layer) instead of co-allocating, freeing 34KB/partition.
  - AG_MODE="chunked" (optional): chunk-major table rows so each per-layer
    AllGather splits into NCH contiguous-slice collectives overlapped with
    the edge pipeline. Measured neutral vs one AG per layer on HW (cost is
    within session noise), so "real" stays the default.
  - v2 retained: sharded node phase + per-layer bf16 table AllGather;
    dst-A from a local a_tab gather; L3 transposed gathers with PE-transpose
    scatter accumulating G^T = sum_e s_e*[h2_e|1] then one G^T.T @ W3x.
"""

import sys

if "/opt/trn_rl_repo" not in sys.path:
    sys.path.insert(0, "/opt/trn_rl_repo")

import numpy as np
import ml_dtypes

import concourse.bacc as bacc
import concourse.bass as bass  # noqa: F401
import concourse.mybir as mybir
import concourse.tile as tile
from concourse.bass_utils import run_bass_kernel_spmd

F32 = mybir.dt.float32
BF16 = mybir.dt.bfloat16
I16 = mybir.dt.int16
AF = mybir.ActivationFunctionType
ALU = mybir.AluOpType
BF = ml_dtypes.bfloat16

P = 128
NCORES = 8
SIM_MODE = False
# "chunked": table rows are chunk-major; each layer's AllGather is split into
#   NCH collectives emitted as soon as their third of the node phase is done,
#   overlapping the collective with the previous layer's edge pipeline.
# "real": single AllGather per layer (core-major rows).
# "none": collectives disabled (timing experiments only — results invalid).
AG_MODE = "real"
GATHER_MODE = "real"  # "none" skips dma_gathers (timing experiments only)
CHB = 10          # bins per AG chunk
NCH = 3           # chunks (= nbc / CHB)
CB12 = 4  # bins per gather chunk, layers 1/2
CB3 = 2   # bins per gather chunk, layer 3


class Cfg:
    def __init__(self, n_real, nbc, dims):
        self.n_real = n_real
        self.nbc = nbc
        self.nb = nbc * NCORES
        self.ng = self.nb * P
        self.dims = dims


CFG = Cfg(30000, 30, [(256, 64), (64, 64), (64, 256)])


# ---------------------------------------------------------------- host prep

def _balance_bins(weight, nb):
    import heapq

    n = weight.shape[0]
    order = np.argsort(-weight, kind="stable")
    bin_of = np.empty(n, np.int32)
    slot_of = np.empty(n, np.int32)
    counts = np.zeros(nb, np.int32)
    heap = [(0, b) for b in range(nb)]
    heapq.heapify(heap)
    for i in order:
        spill = []
        while True:
            load, b = heapq.heappop(heap)
            if counts[b] < P:
                break
            spill.append((load, b))
        for s in spill:
            heapq.heappush(heap, s)
        bin_of[i] = b
        slot_of[i] = counts[b]
        counts[b] += 1
        heapq.heappush(heap, (load + int(weight[i]), b))
    return bin_of, slot_of


def _wrap16(flat_idx):
    n = flat_idx.shape[0]
    a = flat_idx.reshape(n // 16, 16).T.astype(np.int16)
    return np.tile(a, (8, 1))


def _host_prep(x, src, dst, cfg):
    n = cfg.n_real
    loops = np.arange(n, dtype=np.int64)
    src_all = np.concatenate([src, loops])
    dst_all = np.concatenate([dst, loops])

    indeg = np.bincount(dst_all, minlength=n).astype(np.int64)
    bin_of, slot_of = _balance_bins(indeg, cfg.nb)
    g_of = bin_of.astype(np.int64) * P + slot_of

    e_bin = bin_of[dst_all]
    order = np.argsort(e_bin, kind="stable")
    sb = e_bin[order]
    counts = np.bincount(e_bin, minlength=cfg.nb)
    g_pad = int(np.ceil(max(counts.max(), 1) / P) * P)
    starts = np.zeros(cfg.nb, np.int64)
    starts[1:] = np.cumsum(counts)[:-1]
    rank = np.arange(sb.shape[0]) - starts[sb]

    # Table-row id for AllGathered tables. "chunked": chunk-major layout so
    # each per-layer AG splits into NCH contiguous-slice collectives:
    #   row = chunk*(8*CHB*P) + core*(CHB*P) + (t%CHB)*P + slot
    # "real"/"none": core-major (bin*P + slot).
    if AG_MODE == "chunked":
        c_all = bin_of // cfg.nbc
        t_all = bin_of % cfg.nbc
        tblrow_of = (
            (t_all // CHB).astype(np.int64) * (NCORES * CHB * P)
            + c_all.astype(np.int64) * (CHB * P)
            + (t_all % CHB).astype(np.int64) * P
            + slot_of
        )
    else:
        tblrow_of = g_of

    src_g = np.zeros((cfg.nb, g_pad), np.int64)             # pad edges -> row 0
    dst_slot = np.full((cfg.nb, g_pad), 255, np.int64)      # pad -> no match
    src_g[sb, rank] = tblrow_of[src_all[order]]
    dst_slot[sb, rank] = slot_of[dst_all[order]]

    nt_e = g_pad // P
    per_core = []
    for c in range(NCORES):
        bins = slice(c * cfg.nbc, (c + 1) * cfg.nbc)
        sg = src_g[bins]
        ds = dst_slot[bins]
        srcw = np.concatenate([_wrap16(sg[t]) for t in range(cfg.nbc)], axis=1)
        # local A-table row index: t_local*128 + slot (pads -> 0)
        aidx = np.arange(cfg.nbc)[:, None] * P + np.where(ds == 255, 0, ds)
        dstw = np.concatenate([_wrap16(aidx[t]) for t in range(cfg.nbc)], axis=1)
        tloc = aidx // P
        slt = aidx % P
        if AG_MODE == "chunked":
            gidx = (
                (tloc // CHB) * (NCORES * CHB * P)
                + c * (CHB * P)
                + (tloc % CHB) * P
                + slt
            )
        else:
            gidx = aidx + c * cfg.nbc * P
        dstwg = np.concatenate([_wrap16(gidx[t]) for t in range(cfg.nbc)], axis=1)
        dstc = np.concatenate(
            [ds[t].reshape(nt_e, P).T for t in range(cfg.nbc)], axis=1
        ).astype(BF)
        per_core.append((srcw, dstw, dstwg, dstc))

    # own-node features, feature-major, per core: [256, n_loc] bf16
    c_in = cfg.dims[0][0]
    x1t = np.zeros((c_in, cfg.ng), BF)
    x1t[:, g_of] = x.T.astype(BF)
    return g_of, g_pad, per_core, x1t


def _fuse_weights(ws, cfg):
    """Per layer: wmat [ci, 3co] cols = [x'|B|A], brep biases [3co];
    L3 separate: w3B/w3A [64,256]+biases, w3x_aug [65,256] (row 64 = b_lin3)."""
    out = []
    for li, (ci, co) in enumerate(cfg.dims, start=1):
        wl = ws[f"w_lin{li}"].astype(np.float64)
        bl = ws[f"b_lin{li}"].astype(np.float64)
        ws1 = ws[f"w_s1_{li}"].astype(np.float64)
        bs1 = ws[f"b_s1_{li}"].astype(np.float64)
        ws2 = ws[f"w_s2_{li}"].astype(np.float64)
        bs2 = ws[f"b_s2_{li}"].astype(np.float64)
        wi, wj = ws1[:co], ws1[co:]
        wmat = np.zeros((ci, 3 * co), np.float64)
        bias = np.zeros((3 * co,), np.float64)
        wmat[:, :co] = wl
        bias[:co] = bl
        wmat[:, co : 2 * co] = wl @ wj
        bias[co : 2 * co] = bl @ wj
        wmat[:, 2 * co :] = wl @ wi
        bias[2 * co :] = bl @ wi + bs1
        out.append(
            dict(
                wmat=wmat.astype(BF),
                bias=bias.astype(np.float32),
                w2=ws2[:, 0].astype(np.float32),
                b2=np.float32(bs2[0]),
                wlin=wl,
                blin=bl,
            )
        )
    return out


# ---------------------------------------------------------------- program

def _build_program(cfg, g_pad):
    nbc, ng = cfg.nbc, cfg.ng
    nt_e = g_pad // P
    dims = cfg.dims
    n_loc = nbc * P
    co3 = dims[2][1]  # 256

    # bf16 const blob columns
    c_iota, c_ident = 0, 128
    c_w2 = [256, 256 + 64, 256 + 128]  # w2 reps: 64,64,256
    c_brep1 = 256 + 128 + 256          # 192
    c_brep2 = c_brep1 + 192            # 192
    c_brep3b = c_brep2 + 192           # 256 (B3 bias rep)
    c_brep3a = c_brep3b + 256          # 256 (A3 bias rep)
    c_iotarep = c_brep3a + 256         # 128*nt_e: [p, s*nt_e+j] = s
    cb_cols = c_iotarep + P * nt_e

    nc = bacc.Bacc("TRN2", target_bir_lowering=False, debug=False, num_devices=NCORES,
                   dynamic_dma_scratch_size=32768)

    xa1_d = nc.dram_tensor("xa1", [2, P, n_loc], BF16, kind="ExternalInput")
    cstb_d = nc.dram_tensor("cstb", [P, cb_cols], BF16, kind="ExternalInput")
    cstf_d = nc.dram_tensor("cstf", [P, 4], F32, kind="ExternalInput")  # b2 cols x3
    srcw_d = nc.dram_tensor("srcw", [P, nbc * g_pad // 16], I16, kind="ExternalInput")
    dstw_d = nc.dram_tensor("dstw", [P, nbc * g_pad // 16], I16, kind="ExternalInput")
    dstwg_d = nc.dram_tensor("dstwg", [P, nbc * g_pad // 16], I16, kind="ExternalInput")
    dstc_d = nc.dram_tensor("dstc", [P, nbc * nt_e], BF16, kind="ExternalInput")
    w1_d = nc.dram_tensor("w1", [2, P, 192], BF16, kind="ExternalInput")
    w2m_d = nc.dram_tensor("w2m", [64, 192], BF16, kind="ExternalInput")
    w3b_d = nc.dram_tensor("w3b", [P, co3], BF16, kind="ExternalInput")
    w3a_d = nc.dram_tensor("w3a", [P, co3], BF16, kind="ExternalInput")
    w3x_d = nc.dram_tensor("w3x", [65, co3], BF16, kind="ExternalInput")
    out_d = nc.dram_tensor("out", [n_loc, co3], F32, kind="ExternalOutput")

    with tile.TileContext(nc) as tc:
        with (
            tc.tile_pool(name="cst", bufs=1) as cpool,
            tc.tile_pool(name="persist", bufs=1) as ppool,
            tc.tile_pool(name="xa", bufs=2) as xapool,
            tc.tile_pool(name="wrt", bufs=2) as wpool,
            tc.tile_pool(name="stg", bufs=3) as spool,
            tc.tile_pool(name="gath", bufs=2) as gpool,
            tc.tile_pool(name="oh", bufs=3) as opool,
            tc.tile_pool(name="sc", bufs=2) as scpool,
            tc.tile_pool(name="psA", bufs=2, space="PSUM") as pspA,
            tc.tile_pool(name="psB", bufs=2, space="PSUM") as pspB,
            tc.tile_pool(name="dram", bufs=1, space="DRAM") as dpool,
        ):
            # ---------------- constants / persistent inputs
            cstb = cpool.tile([P, cb_cols], BF16)
            nc.sync.dma_start(cstb[:], cstb_d[:])
            cstf = cpool.tile([P, 4], F32)
            nc.sync.dma_start(cstf[:], cstf_d[:])
            srcw = cpool.tile([P, nbc * g_pad // 16], I16)
            dstw = cpool.tile([P, nbc * g_pad // 16], I16)
            dstwg = cpool.tile([P, nbc * g_pad // 16], I16)
            dstc = cpool.tile([P, nbc * nt_e], BF16)

            def load_idx():
                # deferred: needed by edge phases only; loads overlap AG0
                nc.sync.dma_start(srcw[:], srcw_d[:])
                nc.sync.dma_start(dstw[:], dstw_d[:])
                nc.sync.dma_start(dstwg[:], dstwg_d[:])
                nc.sync.dma_start(dstc[:], dstc_d[:])
            w1 = cpool.tile([P, 2, 192], BF16)
            nc.sync.dma_start(w1[:], w1_d[:].rearrange("c p n -> p c n"))
            w2m = cpool.tile([64, 192], BF16)
            nc.sync.dma_start(w2m[:], w2m_d[:])
            w3b = cpool.tile([P, co3], BF16)
            nc.sync.dma_start(w3b[:], w3b_d[:])
            w3a = cpool.tile([P, co3], BF16)
            nc.sync.dma_start(w3a[:], w3a_d[:])
            w3x = cpool.tile([65, co3], BF16)
            nc.sync.dma_start(w3x[:], w3x_d[:])

            iota_row = cstb[:, c_iota : c_iota + P]      # [P,128], row-iota
            identb = cstb[:, c_ident : c_ident + P]      # [P,128] identity bf16

            # xloc: per-layer own output, feature-major, row 64 = ones
            xloc = [ppool.tile([65, n_loc], BF16, tag=f"xloc{l}", name=f"xloc{l}") for l in range(2)]
            nc.vector.memset(xloc[1][64:65, :], 1.0)

            # ---------------- DRAM internals
            tbl = [
                dpool.tile([ng, P], BF16, tag=f"tbl{l}", name=f"tbl{l}",
                           addr_space="Local" if (SIM_MODE or AG_MODE == "chunked") else "Shared")
                for l in range(2)
            ]
            tbl_loc = [dpool.tile([n_loc, P], BF16, tag=f"tbll{l}", name=f"tbll{l}") for l in range(2)]
            hb3 = dpool.tile([ng, P], BF16, tag="hb3", name="hb3",
                             addr_space="Local" if (SIM_MODE or AG_MODE == "chunked") else "Shared")
            hb3_loc = dpool.tile([n_loc, P], BF16, tag="hb3l", name="hb3l")
            atab = [dpool.tile([n_loc, P], BF16, tag=f"atab{l}", name=f"atab{l}") for l in range(2)]

            def allgather(loc, full):
                if AG_MODE == "none":
                    return
                if SIM_MODE:
                    rows = loc.shape[0]
                    for r in range(NCORES):
                        nc.sync.dma_start(full[:][r * rows : (r + 1) * rows, :], loc[:])
                else:
                    nc.gpsimd.collective_compute(
                        "AllGather",
                        ALU.bypass,
                        replica_groups=[list(range(NCORES))],
                        ins=[loc.opt()],
                        outs=[full.opt()],
                    )

            def allgather_chunk(loc, full, k):
                """AG of node-bin chunk k: contiguous row slices on both sides."""
                if AG_MODE == "none":
                    return
                rows = CHB * P
                in_ap = loc[:][k * rows : (k + 1) * rows, :]
                out_ap = full[:][k * NCORES * rows : (k + 1) * NCORES * rows, :]
                if SIM_MODE:
                    for r in range(NCORES):
                        nc.sync.dma_start(
                            full[:][k * NCORES * rows + r * rows :][0:rows, :], in_ap
                        )
                else:
                    nc.gpsimd.collective_compute(
                        "AllGather",
                        ALU.bypass,
                        replica_groups=[list(range(NCORES))],
                        ins=[in_ap],
                        outs=[out_ap],
                    )

            # ================ node phase layer l (own bins): tables + a_tab
            # make_node_emitter(l) returns emit(t): emits bin t's node work.
            # Called standalone (node_phase) or interleaved into the previous
            # layer's edge loop so the node matmuls/stores hide under it.
            def make_node_emitter(l):
                co = dims[l][1]
                state = {}

                def emit(t):
                    cols = slice(t * P, (t + 1) * P)
                    if l == 2:
                        ps_h = pspB.tile([P, P], F32, space="PSUM", tag="g")
                        nc.tensor.matmul(out=ps_h[:], lhsT=xloc[1][:, cols], rhs=identb[0:65, :], start=True, stop=True)
                        hbs = spool.tile([P, P], BF16, tag="hbs")
                        nc.vector.tensor_copy(out=hbs[:], in_=ps_h[:])
                        nc.sync.dma_start(hb3_loc[:][t * P : (t + 1) * P, :], hbs[:])
                        return
                    ps = pspA.tile([P, 192], F32, space="PSUM", tag="big")
                    if l == 0:
                        XB = 10
                        if t % XB == 0:
                            state["xa"] = xapool.tile([P, 2, XB * P], BF16, tag="xa", name="xa")
                            gcols = slice(t * P, min((t + XB) * P, n_loc))
                            nc.sync.dma_start(
                                state["xa"][:, :, 0 : gcols.stop - gcols.start],
                                xa1_d[:][:, :, gcols].rearrange("c p n -> p c n"),
                            )
                        lc = slice((t % XB) * P, (t % XB + 1) * P)
                        for k in range(2):
                            nc.tensor.matmul(
                                out=ps[:], lhsT=state["xa"][:, k, lc], rhs=w1[:, k, :],
                                start=(k == 0), stop=(k == 1),
                            )
                    else:
                        nc.tensor.matmul(out=ps[:], lhsT=xloc[0][0:64, cols], rhs=w2m[:], start=True, stop=True)
                    brep = cstb[:, (c_brep1 if l == 0 else c_brep2) :][:, 0:192]
                    WB = 5
                    g = t % WB
                    if g == 0:
                        state["tbs"] = wpool.tile([P, WB, P], BF16, tag="tbs", name="tbs")
                        state["ats"] = wpool.tile([P, WB, P], BF16, tag="ats", name="ats")
                    tbs, ats = state["tbs"], state["ats"]
                    nc.vector.tensor_tensor(out=tbs[:, g, :], in0=ps[:, 0:P], in1=brep[:, 0:P], op=ALU.add)
                    nc.vector.tensor_tensor(
                        out=ats[:, g, 0:co], in0=ps[:, 2 * co : 3 * co],
                        in1=brep[:, 2 * co : 3 * co], op=ALU.add,
                    )
                    nc.vector.memset(ats[:, g, co:P], 0.0)
                    if g == WB - 1:
                        t0 = t - WB + 1
                        nc.sync.dma_start(
                            tbl_loc[l][:][t0 * P : (t + 1) * P, :]
                            .rearrange("(g p) n -> p g n", p=P),
                            tbs[:],
                        )
                        nc.sync.dma_start(
                            atab[l][:][t0 * P : (t + 1) * P, :]
                            .rearrange("(g p) n -> p g n", p=P),
                            ats[:],
                        )

                return emit

            def node_phase(l, ag=None):
                emit = make_node_emitter(l)
                for t in range(nbc):
                    emit(t)
                    if ag is not None and t % CHB == CHB - 1:
                        allgather_chunk(*ag, t // CHB)

            # ================ edge phase layer l (own bins)
            def edge_phase(l, post_emit=None, ag=None):
                co = dims[l][1]
                w2rep = cstb[:, c_w2[l] : c_w2[l] + co]
                b2col = cstf[:, l : l + 1]
                cb = CB3 if l == 2 else CB12
                nch = (nbc + cb - 1) // cb
                gs = ga = None
                pend = None

                def emit_b3(tb, gstb, tlb, ssigb, ohb):
                    gt_ps = pspB.tile([65, P], F32, space="PSUM", tag="g")

                    def tr_mm(j):
                        esl = slice(tlb * g_pad + j * P, tlb * g_pad + (j + 1) * P)
                        tr = pspB.tile([P, P], F32, space="PSUM", tag="scat")
                        nc.tensor.matmul(out=tr[:], lhsT=gstb[:, 0, esl], rhs=identb, start=True, stop=True)
                        return tr

                    trs = [tr_mm(0), tr_mm(1)]
                    for j in range(nt_e):
                        hs = opool.tile([P, P], BF16, tag="sm")
                        nc.scalar.activation(
                            out=hs[:], in_=trs[j][:], func=AF.Copy,
                            scale=ssigb[:, j : j + 1],
                        )
                        nc.tensor.matmul(
                            out=gt_ps[:], lhsT=hs[:, 0:65], rhs=ohb[:, :, j],
                            start=(j == 0), stop=(j == nt_e - 1),
                        )
                        if j + 2 < nt_e:
                            trs.append(tr_mm(j + 2))
                    gt_sb = spool.tile([65, P], BF16, tag="gt_sb")
                    nc.vector.tensor_copy(out=gt_sb[:], in_=gt_ps[:])
                    o_ps = pspA.tile([P, co3], F32, space="PSUM", tag="obig")
                    nc.tensor.matmul(out=o_ps[:], lhsT=gt_sb[:], rhs=w3x[:], start=True, stop=True)
                    ostg = spool.tile([P, co3], F32, tag="ostg")
                    nc.scalar.activation(out=ostg[:], in_=o_ps[:], func=AF.Copy)
                    nc.sync.dma_start(out_d[tb * P : (tb + 1) * P, :], ostg[:])

                for t in range(nbc):
                    if t % cb == 0:
                        hn = min(cb, nbc - t)
                        ni = hn * g_pad
                        isl = slice(t * g_pad // 16, (t + hn) * g_pad // 16)
                        # unified gather buffers: one flat [P, CB12*nt_e*P] slot
                        # per stream, viewed per-layer (L1/L2 row-major chunks,
                        # L3 transposed edge-major) so L3 doesn't co-allocate.
                        gbuf0 = gpool.tile([P, CB12 * nt_e * P], BF16, tag="g0", name="gbuf0")
                        gbuf1 = gpool.tile([P, CB12 * nt_e * P], BF16, tag="g1", name="gbuf1")
                        if l == 2:
                            gst = gbuf0[:, 0 : CB3 * g_pad].rearrange("p (o e) -> p o e", o=1)
                            gdt = gbuf1[:, 0 : CB3 * g_pad].rearrange("p (o e) -> p o e", o=1)
                            if GATHER_MODE != "none":
                                nc.gpsimd.dma_gather(
                                    out_ap=gst[:, :, 0:ni], in_ap=hb3[:],
                                    idxs_ap=srcw[:, isl], num_idxs=ni, num_idxs_reg=ni,
                                    elem_size=P, transpose=True, single_packet=False,
                                )
                                nc.gpsimd.dma_gather(
                                    out_ap=gdt[:, :, 0:ni], in_ap=hb3[:],
                                    idxs_ap=dstwg[:, isl], num_idxs=ni, num_idxs_reg=ni,
                                    elem_size=P, transpose=True, single_packet=False,
                                )
                        else:
                            gs = gbuf0[:].rearrange("p (j e) -> p j e", e=P)
                            ga = gbuf1[:].rearrange("p (j e) -> p j e", e=P)
                            if GATHER_MODE != "none":
                                nc.gpsimd.dma_gather(
                                    out_ap=ga[:, 0 : hn * nt_e, :], in_ap=atab[l][:],
                                    idxs_ap=dstw[:, isl], num_idxs=ni, num_idxs_reg=ni,
                                    elem_size=P, single_packet=False,
                                )
                                nc.gpsimd.dma_gather(
                                    out_ap=gs[:, 0 : hn * nt_e, :], in_ap=tbl[l][:],
                                    idxs_ap=srcw[:, isl], num_idxs=ni, num_idxs_reg=ni,
                                    elem_size=P, single_packet=False,
                                )
                    tl = t % cb
                    g3 = None if l == 2 else gs[:, tl * nt_e : (tl + 1) * nt_e, :]
                    spre = scpool.tile([P, nt_e], F32, tag="spre")
                    if l == 2:
                        for j in range(nt_e):
                            esl = slice(tl * g_pad + j * P, tl * g_pad + (j + 1) * P)
                            ab_ps = pspA.tile([P, co3], F32, space="PSUM", tag="big")
                            nc.tensor.matmul(out=ab_ps[:], lhsT=gst[:, 0, esl], rhs=w3b[:], start=True, stop=False)
                            nc.tensor.matmul(out=ab_ps[:], lhsT=gdt[:, 0, esl], rhs=w3a[:], start=False, stop=True)
                            scr = scpool.tile([P, co3], F32, tag="scr")
                            nc.vector.scalar_tensor_tensor(
                                out=scr[:], in0=ab_ps[:], scalar=0.0,
                                in1=w2rep, op0=ALU.max, op1=ALU.mult,
                                accum_out=spre[:, j : j + 1],
                            )
                    else:
                        a3 = ga[:, tl * nt_e : (tl + 1) * nt_e, :]
                        ssum = scpool.tile([P, nt_e, co], BF16, tag="ssum")
                        nc.vector.tensor_tensor(
                            out=ssum[:], in0=g3[:, :, 64:P], in1=a3[:, :, 0:co], op=ALU.add,
                        )
                        # batched relu*w2 over all nt_e tiles, then segmented sum
                        scr = scpool.tile([P, nt_e, co], F32, tag="scr")
                        nc.vector.scalar_tensor_tensor(
                            out=scr[:], in0=ssum[:], scalar=0.0,
                            in1=w2rep.rearrange("p (o c) -> p o c", o=1)
                            .to_broadcast([P, nt_e, co]),
                            op0=ALU.max, op1=ALU.mult,
                        )
                        nc.vector.tensor_reduce(
                            out=spre[:], in_=scr[:], axis=mybir.AxisListType.X, op=ALU.add,
                        )
                    ssig = scpool.tile([P, nt_e], F32, tag="ssig")
                    nc.scalar.activation(out=ssig[:], in_=spre[:], func=AF.Sigmoid, bias=b2col)

                    # one-hot [e, slot, j] (packed operands -> DVE 2x mode)
                    oh = opool.tile([P, P, nt_e], BF16, tag="oh")
                    nc.vector.tensor_tensor(
                        out=oh[:],
                        in0=dstc[:, t * nt_e : (t + 1) * nt_e]
                        .rearrange("p (o j) -> p o j", o=1).to_broadcast([P, P, nt_e]),
                        in1=cstb[:, c_iotarep : c_iotarep + P * nt_e].rearrange(
                            "p (s j) -> p s j", s=P),
                        op=ALU.is_equal,
                    )

                    if l == 2:
                        if pend is not None:
                            emit_b3(*pend)
                        pend = (t, gst, tl, ssig, oh)
                    else:
                        # per-j score scaling on ACT (idle engine; DVE is the
                        # edge-phase bottleneck and broadcast muls run at 1x)
                        s_ps = pspB.tile([64, P], F32, space="PSUM", tag="scat")
                        for j in range(nt_e):
                            msg = opool.tile([P, co], BF16, tag="sm")
                            nc.scalar.activation(
                                out=msg[:], in_=g3[:, j, 0:co], func=AF.Copy,
                                scale=ssig[:, j : j + 1],
                            )
                            nc.tensor.matmul(
                                out=s_ps[:], lhsT=msg[:], rhs=oh[:, :, j],
                                start=(j == 0), stop=(j == nt_e - 1),
                            )
                        nc.scalar.activation(
                            out=xloc[l][0:64, t * P : (t + 1) * P], in_=s_ps[:], func=AF.Relu
                        )
                    if post_emit is not None:
                        post_emit(t)
                        if ag is not None and t % CHB == CHB - 1:
                            allgather_chunk(*ag, t // CHB)
                if l == 2 and pend is not None:
                    emit_b3(*pend)

            # ================ schedule
            # node phases for layers 2/3 are interleaved bin-by-bin into the
            # previous layer's edge loop so they hide under the edge pipeline.
            if AG_MODE == "chunked":
                node_phase(0, ag=(tbl_loc[0], tbl[0]))
                load_idx()
                edge_phase(0, post_emit=make_node_emitter(1), ag=(tbl_loc[1], tbl[1]))
                edge_phase(1, post_emit=make_node_emitter(2), ag=(hb3_loc, hb3))
                edge_phase(2)
            else:
                node_phase(0)
                load_idx()
                allgather(tbl_loc[0], tbl[0])
                edge_phase(0, post_emit=make_node_emitter(1))
                allgather(tbl_loc[1], tbl[1])
                edge_phase(1, post_emit=make_node_emitter(2))
                allgather(hb3_loc, hb3)
                edge_phase(2)

    nc.compile()
    return nc


# ---------------------------------------------------------------- driver

_PROG_CACHE = {}


def _make_in_maps(inputs, cfg, g_pad, per_core, x1t, fw):
    nbc = cfg.nbc
    n_loc = nbc * P
    co3 = cfg.dims[2][1]

    nt_e = g_pad // P
    cb_cols = 256 + 128 + 256 + 192 + 192 + 256 + 256 + P * nt_e
    cstb = np.zeros((P, cb_cols), BF)
    cstb[:, 0:128] = np.arange(128, dtype=np.float32)[None, :].astype(BF)
    cstb[:, 128:256] = np.eye(128, dtype=np.float32).astype(BF)
    off = 256
    for l in range(3):
        co = cfg.dims[l][1]
        cstb[:, off + (0 if l == 0 else (64 if l == 1 else 128)) :][:, 0:co] = (
            fw[l]["w2"][None, :].astype(BF)
        )
    off = 256 + 128 + 256
    cstb[:, off : off + 192] = fw[0]["bias"][None, :].astype(BF)
    cstb[:, off + 192 : off + 384] = fw[1]["bias"][None, :].astype(BF)
    cstb[:, off + 384 : off + 384 + 256] = fw[2]["bias"][None, co3 : 2 * co3].astype(BF)
    cstb[:, off + 640 : off + 640 + 256] = fw[2]["bias"][None, 2 * co3 : 3 * co3].astype(BF)
    nt_e = g_pad // P
    c_iotarep = off + 640 + 256
    cstb[:, c_iotarep : c_iotarep + P * nt_e] = np.repeat(
        np.arange(P, dtype=np.float32), nt_e
    )[None, :].astype(BF)

    cstf = np.zeros((P, 4), np.float32)
    for l in range(3):
        cstf[:, l] = fw[l]["b2"]

    w1 = np.zeros((2, P, 192), BF)
    w1[0] = fw[0]["wmat"][0:128]
    w1[1] = fw[0]["wmat"][128:256]
    w2m = np.ascontiguousarray(fw[1]["wmat"])
    w3b = np.zeros((P, co3), BF)
    w3b[0:64] = fw[2]["wmat"][:, co3 : 2 * co3]
    w3b[64] = fw[2]["bias"][co3 : 2 * co3].astype(BF)
    w3a = np.zeros((P, co3), BF)
    w3a[0:64] = fw[2]["wmat"][:, 2 * co3 : 3 * co3]
    w3a[64] = fw[2]["bias"][2 * co3 : 3 * co3].astype(BF)
    w3x = np.zeros((65, co3), BF)
    w3x[0:64] = fw[2]["wlin"].astype(BF)
    w3x[64] = fw[2]["blin"].astype(BF)

    in_maps = []
    for c in range(NCORES):
        srcw, dstw, dstwg, dstc = per_core[c]
        xa1 = np.zeros((2, P, n_loc), BF)
        xa1[0] = x1t[0:128, c * n_loc : (c + 1) * n_loc]
        xa1[1] = x1t[128:256, c * n_loc : (c + 1) * n_loc]
        in_maps.append(
            {
                "xa1": xa1,
                "cstb": cstb,
                "cstf": cstf,
                "srcw": srcw,
                "dstw": dstw,
                "dstwg": dstwg,
                "dstc": dstc,
                "w1": w1,
                "w2m": w2m,
                "w3b": w3b,
                "w3a": w3a,
                "w3x": w3x,
            }
        )
    return in_maps


def _run(inputs, cfg, trace=False):
    x = np.ascontiguousarray(np.asarray(inputs["x"], dtype=np.float32))
    ei = np.asarray(inputs["edge_index"]).astype(np.int64)
    src, dst = ei[0], ei[1]

    g_of, g_pad, per_core, x1t = _host_prep(x, src, dst, cfg)
    fw = _fuse_weights(inputs, cfg)

    key = (cfg.n_real, cfg.nbc, g_pad)
    if key not in _PROG_CACHE:
        _PROG_CACHE[key] = _build_program(cfg, g_pad)
    nc = _PROG_CACHE[key]

    in_maps = _make_in_maps(inputs, cfg, g_pad, per_core, x1t, fw)
    res = run_bass_kernel_spmd(nc, in_maps, core_ids=list(range(NCORES)), trace=trace)

    n_loc = cfg.nbc * P
    full = np.empty((cfg.ng, cfg.dims[-1][1]), np.float32)
    for c in range(NCORES):
        full[c * n_loc : (c + 1) * n_loc] = res.results[c]["out"]
    out = full[g_of]
    return out, res


def kernel(**inputs) -> np.ndarray:
    out, _ = _run(inputs, CFG, trace=False)
    return out

